# revision 21
# baseline (speedup 1.0000x reference)
"""Trainium2 Bass kernel for nn_MixOp (hard gumbel-softmax routed conv+BN+ReLU).

Forward semantics (from the reference):
  index  = argmax(softmax((logits + g) / TAU))            # routing, 5 branches
  y      = relu(conv(x, w[index]) * inv + (beta - mean*inv))   for that branch
  out    = y * take(onehot + soft - stop_grad(soft), index) == y * 1.0  (exact)

Only the selected branch runs.  Routing is evaluated on host (5 scalars,
mirroring the reference's lax.switch dispatch); the conv+BN+ReLU runs on 8
NeuronCores, data-parallel over batch (4 images per core).

Per-core conv formulation: for each output channel `co` the KxK conv is a sum
over (kw, ci) of 1-D convolutions along H.  Each 1-D H-conv is one matmul on
the PE array:
    stationary lhsT = Toeplitz band T[hi, ho] = w[hi-ho, kw, ci, co]   (128 x HO)
    moving rhs      = x_tile[:, kw : kw+512, ci]                       (128 x 512)
accumulated over the K*cin_g (kw, ci) passes in one PSUM bank.  H is tiled in
bands of HO = 128 - 2*pad output rows; the ragged last rows of all 4 images
are packed block-diagonally into one extra "tail" matmul set.  Zero padding
(SAME) is pre-applied on the host so every SBUF x-tile is written by exactly
one DMA.  BN+ReLU is fused into the PSUM->SBUF eviction on the scalar engine
(relu(scale*x+bias)), writing channel-strided into an NHWC tile that DMAs out
as contiguous rows.

Precision modes (fp32 PE matmul is ~10 cyc/col on TRN2 -- avoid):
  fp16x3 (default): x and the Toeplitz weights are split into fp16 hi+lo
     halves (x = x_hi + x_lo exactly to ~2^-22).  Each logical matmul becomes
     3 fp16 matmuls (x_hi*w_hi + x_lo*w_hi + x_hi*w_lo) accumulating in the
     fp32 PSUM -> fp32-class accuracy (~1e-6 rel) at 3x fp16 speed.
  fp16: single-pass fp16 (~3e-4 rel err), ~3x faster still.
  bf16/fp32: kept for experiments.
"""

import os
import sys

import numpy as np

for _p in ("/opt/trn_rl_repo",):
    if _p not in sys.path and os.path.isdir(_p):
        sys.path.insert(0, _p)

TAU = 1.0
EPS = 1e-5
GROUPS = (1, 1, 4, 1, 4)
KSIZES = (1, 3, 3, 5, 5)
B, H, W, C = 32, 512, 512, 4
N_CORES = 8
B_SH = B // N_CORES  # images per core

MODE = os.environ.get("MIXOP_MODE", "fp16x3")

# Stash of the last BassKernelResults (exec_time_ns etc.) for the local harness.
LAST_RESULTS = None


def _ensure_ntff_hook():
    """Make `antenv.axon_hooks` importable so run_bass_kernel_spmd(trace=True)
    can NTFF-profile under axon (or degrade gracefully instead of crashing)."""
    import types
    import contextlib
    import ctypes

    try:
        import antenv.axon_hooks  # noqa: F401

        return
    except ImportError:
        pass
    try:
        import antenv
    except ImportError:
        return
    mod = types.ModuleType("antenv.axon_hooks")
    _hook = [None]
    mod.set_axon_ntff_profile_hook = lambda h: _hook.__setitem__(0, h)
    mod.get_axon_ntff_profile_hook = lambda: _hook[0]
    sys.modules["antenv.axon_hooks"] = mod
    antenv.axon_hooks = mod

    so_path = "/opt/axon/libaxon_pjrt.so"
    if not os.path.exists(so_path):
        return
    try:
        lib = ctypes.CDLL(so_path)
        if not hasattr(lib, "axon_start_nrt_profile"):
            return
        lib.axon_start_nrt_profile.argtypes = [
            ctypes.POINTER(ctypes.c_int64),
            ctypes.c_size_t,
        ]
        lib.axon_start_nrt_profile.restype = ctypes.c_int64
        lib.axon_stop_nrt_profile.argtypes = [ctypes.c_char_p]
        lib.axon_stop_nrt_profile.restype = ctypes.c_int64

        @contextlib.contextmanager
        def _ntff_hook(output_dir, device_ids):
            import jax

            jax.devices()
            if device_ids:
                ids = (ctypes.c_int64 * len(device_ids))(*device_ids)
                rc = lib.axon_start_nrt_profile(ids, len(device_ids))
            else:
                rc = lib.axon_start_nrt_profile(None, 0)
            if rc != 0:
                raise RuntimeError(f"axon_start_nrt_profile rc={rc}")
            try:
                yield
            finally:
                n = lib.axon_stop_nrt_profile(str(output_dir).encode())
                print(f"ntff profile: {n} file(s) written to {output_dir}")

        mod.set_axon_ntff_profile_hook(_ntff_hook)
    except Exception:
        pass


def _routing_index(logits, g):
    s = (np.asarray(logits, np.float32) + np.asarray(g, np.float32)) / np.float32(TAU)
    e = np.exp(s - s.max())
    soft = e / e.sum()
    return int(np.argmax(soft))


def _mode_config():
    """-> (np_dt, mybir dt name, XD, terms [(x_half, w_half)])."""
    if MODE == "fp32":
        return np.float32, "float32", 1, [(0, 0)]
    if MODE == "bf16":
        import ml_dtypes

        return ml_dtypes.bfloat16, "bfloat16", 1, [(0, 0)]
    if MODE == "fp16":
        return np.float16, "float16", 1, [(0, 0)]
    if MODE == "fp16x3":
        return np.float16, "float16", 2, [(0, 0), (1, 0), (0, 1)]
    raise ValueError(MODE)


def _build_toeplitz(w, K, groups, HO, ho_rem, inv):
    """Host-built fp32 stationary stacks, with the BN scale inv[co] folded in.

    Returns (tfull [128, S, HO], ttail [128, S, 4*ho_rem] | None,
             pairs: per-co list of (kw, ci_moving) in stationary order).
    """
    cin_g = C // groups
    S = 4 * K * cin_g

    tfull = np.zeros((128, S, HO), np.float32)
    ttail = np.zeros((128, S, 4 * ho_rem), np.float32) if ho_rem else None
    pairs = []
    jo = np.arange(HO)
    jt = np.arange(ho_rem)
    s = 0
    for co in range(4):
        plist = []
        for kw in range(K):
            for ci in range(cin_g):
                plist.append((kw, co if groups == 4 else ci))
                for kh in range(K):
                    wv = np.float32(
                        np.float32(w[kh, kw, 0 if groups == 4 else ci, co])
                        * np.float32(inv[co])
                    )
                    tfull[jo + kh, s, jo] = wv
                    if ttail is not None:
                        for i in range(4):
                            ttail[32 * i + jt + kh, s, ho_rem * i + jt] = wv
                s += 1
        pairs.append(plist)
    assert s == S
    return tfull, ttail, pairs


def _hilo(a32, np_dt, XD):
    """[..., D] fp32 -> [..., XD, D] in np_dt (hi, and residual lo if XD=2)."""
    hi = a32.astype(np_dt)
    if XD == 1:
        return hi[..., None, :]
    lo = (a32 - hi.astype(np.float32)).astype(np_dt)
    return np.stack([hi, lo], axis=-2)


def _build_program(K, pairs, S, HO, ho_rem, inv, bvec, dt_name, XD, terms):
    import concourse.bacc as bacc
    import concourse.mybir as mybir
    import concourse.tile as tile
    from contextlib import ExitStack

    dt_in = getattr(mybir.dt, dt_name)
    pad = K // 2
    WP = W + 2 * pad  # padded width
    HP = H + 2 * pad  # padded height
    relu = mybir.ActivationFunctionType.Relu

    nc = bacc.Bacc()
    xx = nc.declare_dram_parameter("xpad", [B_SH, HP, XD, C, WP], dt_in, isOutput=False)
    tf = nc.declare_dram_parameter("tfull", [128, S, XD, 128], dt_in, isOutput=False)
    xt_d = tt = None
    if ho_rem:
        xt_d = nc.declare_dram_parameter("xtail", [128, XD, C, WP], dt_in, isOutput=False)
        tt = nc.declare_dram_parameter(
            "ttail", [128, S, XD, 128], dt_in, isOutput=False
        )
    yy = nc.declare_dram_parameter("y", [B_SH, H, W, C], mybir.dt.float32, isOutput=True)

    with tile.TileContext(nc) as tc, ExitStack() as ctx:
        singles = ctx.enter_context(tc.tile_pool(name="singles", bufs=1))
        xpool = ctx.enter_context(tc.tile_pool(name="xpool", bufs=8))
        ypool = ctx.enter_context(tc.tile_pool(name="ypool", bufs=6))
        tailpool = ctx.enter_context(tc.tile_pool(name="tailpool", bufs=1))
        pspool = ctx.enter_context(tc.tile_pool(name="pspool", bufs=8, space="PSUM"))

        bias_sb = singles.tile([128, 4], mybir.dt.float32)
        for co in range(4):
            nc.vector.memset(bias_sb[:, co : co + 1], float(bvec[co]))

        t_sb = singles.tile([128, S, XD, 128], dt_in)
        nc.sync.dma_start(out=t_sb, in_=tf[:, :, :, :])
        tt_sb = None
        xtail_t = None
        if ho_rem:
            tt_sb = singles.tile([128, S, XD, 128], dt_in)
            nc.sync.dma_start(out=tt_sb, in_=tt[:, :, :, :])
            # prefetch the packed tail rows now -- its matmuls run last, but the
            # load has no dependencies and must not queue behind the y-out DMAs
            xtail_t = tailpool.tile([128, XD, C, WP], dt_in, tag="xt")
            nc.gpsimd.dma_start(out=xtail_t, in_=xt_d[:, :, :])

        def do_co(psum_t, x_t, co, n_out, lhs_tile):
            plist = pairs[co]
            n = len(plist)
            nmm = n * len(terms)
            m = 0
            for t, (kw, ci) in enumerate(plist):
                for xh, wh in terms:
                    nc.tensor.matmul(
                        out=psum_t[0:128, 0:512],
                        lhsT=lhs_tile[:, co * n + t, wh, :],
                        rhs=x_t[:, xh, ci, kw : kw + W],
                        start=(m == 0),
                        stop=(m == nmm - 1),
                    )
                    m += 1

        def evict(psum_t, y_t, co, n_out, on_vector):
            if on_vector:
                nc.vector.tensor_scalar(
                    out=y_t[0:n_out, :, co],
                    in0=psum_t[0:n_out, 0:512],
                    scalar1=float(bvec[co]),
                    scalar2=0.0,
                    op0=mybir.AluOpType.add,
                    op1=mybir.AluOpType.max,
                )
            else:
                nc.scalar.activation(
                    out=y_t[0:n_out, :, co],
                    in_=psum_t[0:n_out, 0:512],
                    func=relu,
                    scale=1.0,
                    bias=bias_sb[0:n_out, co : co + 1],
                )

        def do_tail():
            y_t = tailpool.tile([128, W, C], mybir.dt.float32, tag="yt")
            for co in range(4):
                psum_t = pspool.tile([128, 512], mybir.dt.float32, tag="ps")
                do_co(psum_t, xtail_t, co, 4 * ho_rem, tt_sb)
                evict(psum_t, y_t, co, 4 * ho_rem, co % 2 == 1)
            for i in range(B_SH):
                nc.gpsimd.dma_start(
                    out=yy[i, 4 * HO : H, :, :],
                    in_=y_t[ho_rem * i : ho_rem * (i + 1), :, :],
                )

        for img in range(B_SH):
            x_tiles = []
            for b in range(4):
                x_t = xpool.tile([128, XD, C, WP], dt_in, tag="x")
                eng = nc.sync if b % 2 == 0 else nc.scalar
                eng.dma_start(out=x_t, in_=xx[img, b * HO : b * HO + 128, :, :, :])
                x_tiles.append(x_t)

            for b in range(4):
                y_t = ypool.tile([128, W, C], mybir.dt.float32, tag="y")
                on_vec = (img * 4 + b) % 2 == 1
                for co in range(4):
                    psum_t = pspool.tile([128, 512], mybir.dt.float32, tag="ps")
                    do_co(psum_t, x_tiles[b], co, HO, t_sb)
                    evict(psum_t, y_t, co, HO, on_vec)
                nc.gpsimd.dma_start(
                    out=yy[img, b * HO : (b + 1) * HO, :, :], in_=y_t[0:HO, :, :]
                )

            # interleave the packed-tail job mid-kernel where PE has no idle
            if img == 0 and ho_rem:
                do_tail()

    nc.compile()
    return nc


def kernel(**inputs):
    global LAST_RESULTS
    from concourse.bass_utils import run_bass_kernel_spmd

    x = np.asarray(inputs["x"], np.float32)
    index = _routing_index(inputs["logits"], inputs["g"])
    w = np.asarray(inputs[f"w{index}"], np.float32)
    gamma = np.asarray(inputs["gamma"], np.float32)[index]
    beta = np.asarray(inputs["beta"], np.float32)[index]
    mean = np.asarray(inputs["mean"], np.float32)[index]
    var = np.asarray(inputs["var"], np.float32)[index]

    inv = (gamma * (1.0 / np.sqrt(var + np.float32(EPS)))).astype(np.float32)
    bvec = (beta - mean * inv).astype(np.float32)

    K = KSIZES[index]
    groups = GROUPS[index]
    pad = K // 2
    HO = 128 - 2 * pad
    ho_rem = H - 4 * HO
    hin_rem = ho_rem + 2 * pad
    S = 4 * K * (C // groups)

    np_dt, dt_name, XD, terms = _mode_config()

    tfull32, ttail32, pairs = _build_toeplitz(w, K, groups, HO, ho_rem, inv)

    def _pad_stat(t32):
        # [128, S, D] -> hi/lo split, padded to 128 cols: [128, S, XD, 128]
        thl = _hilo(t32, np_dt, XD)
        out = np.zeros((128, S, XD, 128), np_dt)
        out[:, :, :, : t32.shape[2]] = thl
        return np.ascontiguousarray(out)

    tfull = _pad_stat(tfull32)
    ttail = _pad_stat(ttail32) if ttail32 is not None else None

    WP, HP = W + 2 * pad, H + 2 * pad
    nc = _build_program(K, pairs, S, HO, ho_rem, inv, bvec, dt_name, XD, terms)

    xhl = _hilo(x, np_dt, XD)  # [B, H, W, XD, C]
    # planar per-core layout: [B_SH, HP, XD, C, WP]
    xpl = np.ascontiguousarray(np.transpose(xhl, (0, 1, 3, 4, 2)))  # [B,H,XD,C,W]
    in_maps = []
    for c in range(N_CORES):
        xpad = np.zeros((B_SH, HP, XD, C, WP), np_dt)
        xpad[:, pad : pad + H, :, :, pad : pad + W] = xpl[c * B_SH : (c + 1) * B_SH]
        m = {"xpad": xpad, "tfull": tfull}
        if ho_rem:
            xtail = np.zeros((128, XD, C, WP), np_dt)
            for i in range(B_SH):
                xtail[32 * i : 32 * i + hin_rem] = xpad[i, 4 * HO : 4 * HO + hin_rem]
            m["xtail"] = xtail
            m["ttail"] = ttail
        in_maps.append(m)

    _ensure_ntff_hook()
    res = run_bass_kernel_spmd(nc, in_maps, core_ids=list(range(N_CORES)))
    LAST_RESULTS = res
    y = np.concatenate([res.results[c]["y"] for c in range(N_CORES)], axis=0)
    return y
